# revision 45
# baseline (speedup 1.0000x reference)
"""Distributed GCN (3-layer CNF dynamics GNN) on 8 Trainium2 NeuronCores.

Math (per reference):
    gcn(x) = D^-1/2 (A + I) D^-1/2 (x W) + b  with self-loop weight 1
    h0 = relu(bn(gcn0(z)));  h1 = relu(bn(gcn1(h0)));  out = gcn2(h1)

Sharding: nodes are split contiguously across the 8 cores (6250 each); edges
are owned by the dst core.  Per layer, each core computes xw for its local
nodes, scales rows by dinv (folds the src-side normalization) and all-gathers
the scaled rows so every core holds the full message table y in HBM.  The
all-gather is split into SPLITS sub-gathers over node sub-ranges (block-cyclic
tables, gather indices remapped host-side) so edge processing on sub-range 0
overlaps the remaining sub-gathers.  Each core gathers y[src] rows for its
edges via SWDGE dma_gather (<=1024 descriptors per call: hard ring limit) and
reduces them per dst-node tile with a one-hot selection-matrix matmul on the
PE: for a chunk of 128 edges, S[e, m] = ew[e] * (dst_local[e] == m), and PSUM
accumulates  agg[m, :] += S^T @ msgs.  Layers 0/1 run the message path in
bf16 (y, S, gathered msgs; f32 PSUM accumulate); layer 2 stays f32 (the
256B-per-descriptor DMA floor forbids 64-wide bf16 rows).  The dst-side
normalization, self-loop term and batchnorm (stats via ones-matmul +
AllReduce; apply fused into one scalar-engine relu-affine in transposed
layout) follow per node tile.

All edge bookkeeping (chunk grid, padding, gather index layout) is pure
integer restructuring done host-side in numpy; all float math is on device.
"""

import math
import os

import numpy as np

import concourse.bacc as bacc
import concourse.bass as bass
import concourse.mybir as mybir
import concourse.tile as tile
from concourse.bass_utils import run_bass_kernel_spmd

P = 128
NCORES = 8
SPLITS = 2              # all-gather split (node sub-ranges per rank)
CALLC = 8               # 128-edge chunks per dma_gather call (<=1024 descs)
MSG_BUFS = 10
S_BUFS = 24
BN_EPS = 1e-5

LAST_RESULTS = None     # test harness peeks exec_time_ns here

f32 = mybir.dt.float32
bf16 = mybir.dt.bfloat16
i16 = mybir.dt.int16
ALU = mybir.AluOpType
ACTF = mybir.ActivationFunctionType
USE_BF16 = os.environ.get("NOBF16", "") != "1"


def _to_bf16(a):
    import ml_dtypes
    return np.asarray(a, dtype=ml_dtypes.bfloat16)


def _edge_structure(src, dst, ew, n_nodes):
    """Host-side integer restructuring: per-core padded edge streams.

    Nodes are range-partitioned over cores; each core's local node range is
    further cut into SPLITS sub-ranges.  The gather table for sub-range q is
    the block-cyclic concat of every rank's q-th sub-range, so gather index
    = rank * qsize[q] + (local - qoff[q])  (always < NCORES*qsize => int16).

    Returns (shared, per_core): `shared` is the chunk grid (identical across
    cores — one SPMD program), `per_core` the padded data arrays.
    """
    nloc = n_nodes // NCORES
    ntiles = math.ceil(nloc / P)
    if ntiles >= 4:
        # tile-aligned sub-ranges so no tile straddles a gather sub-table
        qt0 = (ntiles + 1) // 2
        qsizes = [qt0 * P, nloc - qt0 * P]
    else:
        qsizes = [len(a) for a in np.array_split(np.arange(nloc), SPLITS)]
    qoff = np.concatenate([[0], np.cumsum(qsizes)])  # [SPLITS+1]
    assert NCORES * max(qsizes) < 32768, "gather index must fit int16"
    core_of = dst // nloc

    percore_raw = []
    counts = np.zeros((NCORES, ntiles, SPLITS), np.int64)
    for c in range(NCORES):
        m = core_of == c
        s_c = src[m]
        d_c = dst[m] - c * nloc
        w_c = ew[m]
        t_c = d_c // P
        sr = s_c // nloc                       # src owner rank
        sl = s_c % nloc                        # src local row
        q_c = np.searchsorted(qoff[1:-1], sl, side="right")
        gidx = sr * np.asarray(qsizes)[q_c] + (sl - qoff[q_c])
        np.add.at(counts[c], (t_c, q_c), 1)
        percore_raw.append((gidx, d_c, w_c, t_c, q_c))

    K = np.ceil(counts / P).astype(np.int64).max(axis=0)  # [ntiles, SPLITS]
    totch = int(K.sum())

    # stream order: q-major, tiles ascending within q
    chunk_of_bucket = {}
    gk = 0
    stream = []
    for q in range(SPLITS):
        for t in range(ntiles):
            chunk_of_bucket[(t, q)] = gk
            for _ in range(int(K[t, q])):
                stream.append((t, q))
                gk += 1
    assert gk == totch

    # dma_gather calls: consecutive chunks of one sub-range, up to CALLC each
    calls = []
    gk = 0
    for q in range(SPLITS):
        nchunks_q = int(K[:, q].sum())
        done = 0
        while done < nchunks_q:
            n = min(CALLC, nchunks_q - done)
            calls.append((gk, n, q))
            gk += n
            done += n
    chunk_call = {}
    for ci, (ck0, n, _q) in enumerate(calls):
        for j in range(n):
            chunk_call[ck0 + j] = (ci, j)

    per_core = []
    maxdeg = 0
    for c in range(NCORES):
        gidx, d_c, w_c, t_c, q_c = percore_raw[c]
        ew_seq = np.zeros(totch * P, np.float32)
        dm_seq = np.zeros(totch * P, np.float32)
        ix_seq = np.zeros(totch * P, np.int16)
        order = np.lexsort((t_c, q_c))
        g_o, d_o, w_o = gidx[order], d_c[order], w_c[order]
        pos = 0
        for q in range(SPLITS):
            for t in range(ntiles):
                n = int(counts[c, t, q])
                if n == 0:
                    continue
                slc = slice(pos, pos + n)
                base = chunk_of_bucket[(t, q)] * P
                ew_seq[base:base + n] = w_o[slc]
                dm_seq[base:base + n] = (d_o[slc] % P).astype(np.float32)
                ix_seq[base:base + n] = g_o[slc].astype(np.int16)
                pos += n
        assert pos == len(d_c)
        ewT = np.ascontiguousarray(ew_seq.reshape(totch, P).T)
        dmT = np.ascontiguousarray(dm_seq.reshape(totch, P).T)
        idx16 = np.ascontiguousarray(ix_seq.reshape(totch * 8, 16).T)
        idxL = np.tile(idx16, (8, 1))

        degs = np.bincount(d_c, minlength=ntiles * P)
        maxdeg = max(maxdeg, int(degs.max(initial=0)))
        per_core.append({"ewT": ewT, "dmT": dmT, "idxL": idxL, "d_c": d_c,
                         "w_c": w_c})

    degcol = max(8, math.ceil(maxdeg / 8) * 8)
    for c in range(NCORES):
        pc = per_core[c]
        d_c, w_c = pc.pop("d_c"), pc.pop("w_c")
        ewbd = np.zeros((ntiles * P, degcol), np.float32)
        if len(d_c):
            o = np.argsort(d_c, kind="stable")
            ds, ws = d_c[o], w_c[o]
            starts = np.zeros(ntiles * P + 1, np.int64)
            np.cumsum(np.bincount(ds, minlength=ntiles * P), out=starts[1:])
            slot = np.arange(len(ds)) - starts[ds]
            ewbd[ds, slot] = ws
        pc["ewbd"] = np.ascontiguousarray(
            ewbd.reshape(ntiles, P, degcol).transpose(1, 0, 2)
            .reshape(P, ntiles * degcol))

    chunks_of_tile = {(t, q): [] for t in range(ntiles) for q in range(SPLITS)}
    for gk, (t, q) in enumerate(stream):
        chunks_of_tile[(t, q)].append(gk)

    # per-tile y-row DMA segments: (row_lo, row_hi, q, offset_in_agin_q)
    ysegs = []
    for t in range(ntiles):
        r0 = t * P
        rows = min(P, nloc - r0)
        segs = []
        for q in range(SPLITS):
            a = max(r0, int(qoff[q]))
            b = min(r0 + rows, int(qoff[q + 1]))
            if a < b:
                segs.append((a - r0, b - r0, q, a - int(qoff[q])))
        ysegs.append(segs)

    shared = {"nloc": nloc, "ntiles": ntiles, "totch": totch, "K": K,
              "calls": calls, "chunk_call": chunk_call,
              "chunks_of_tile": chunks_of_tile, "degcol": degcol,
              "qsizes": qsizes, "ysegs": ysegs}
    return shared, per_core


def _balance_perm(src, dst, n_nodes):
    """Within-core node permutation equalizing per-tile in-edge counts.

    Best-fit-decreasing: pack nodes into tiles so every regular tile's
    in-edge total is <= 16*P (=> 16 chunks, no ceil waste); the remainder
    concentrates in tile 0 (same index on every core, so the max-over-cores
    chunk grid stays tight).  Returns perm with perm[orig_id] = new_id;
    node stays on its original core, only the local row changes.
    """
    nloc = n_nodes // NCORES
    nt = math.ceil(nloc / P)
    qt0 = (nt + 1) // 2
    perm = np.empty(n_nodes, np.int64)
    ecap = 8 * P                     # per-(tile, src-half) chunk budget
    # provisional src-half of every node (kept exact: packing never moves a
    # node across the half boundary)
    half = (np.arange(n_nodes) % nloc) >= qt0 * P
    for c in range(NCORES):
        lo, hi = c * nloc, (c + 1) * nloc
        m = (dst >= lo) & (dst < hi)
        d_loc = dst[m] - lo
        # per-node in-degree split by src half
        sh_edges = half[src[m]]
        d0 = np.bincount(d_loc[~sh_edges], minlength=nloc)
        d1 = np.bincount(d_loc[sh_edges], minlength=nloc)
        newloc = np.empty(nloc, np.int64)
        for hsel, t_lo, t_hi, ovf in ((False, 0, qt0, 0),
                                      (True, qt0, nt, qt0)):
            ids = np.where(half[lo:hi] == hsel)[0]
            dg0, dg1 = d0[ids], d1[ids]
            order = np.argsort(-(dg0 + dg1), kind="stable")
            ntl = t_hi - t_lo
            caps = np.full(ntl, P, np.int64)
            if t_hi == nt:
                caps[-1] = nloc - (nt - 1) * P
            l0 = np.zeros(ntl, np.int64)
            l1 = np.zeros(ntl, np.int64)
            used = np.zeros(ntl, np.int64)
            for j in order:
                a, b = dg0[j], dg1[j]
                free = used < caps
                fit = free & (l0 + a <= ecap) & (l1 + b <= ecap)
                fit[ovf - t_lo] = False
                if fit.any():
                    t = int(np.argmax(np.where(fit, l0 + l1, -1)))
                elif free[ovf - t_lo]:
                    t = ovf - t_lo
                else:
                    t = int(np.argmax(np.where(free, -(l0 + l1),
                                               -(1 << 40))))
                newloc[ids[j]] = (t_lo + t) * P + used[t]
                used[t] += 1
                l0[t] += a
                l1[t] += b
        perm[lo:hi] = lo + newloc
    return perm


def _build_program(n_nodes, d_in, d_hid, shared, no_collectives=False):
    """Emit the SPMD Bass program (same for every core)."""
    nloc = shared["nloc"]
    nt = shared["ntiles"]
    totch = shared["totch"]
    calls = shared["calls"]
    chunk_call = shared["chunk_call"]
    cot = shared["chunks_of_tile"]
    degcol = shared["degcol"]
    qsizes = shared["qsizes"]
    ysegs = shared["ysegs"]
    last_rows = nloc - (nt - 1) * P
    dims = [(d_in, d_hid), (d_hid, d_hid), (d_hid, d_in)]

    nc = bacc.Bacc("TRN2", target_bir_lowering=False, debug=False,
                   num_devices=NCORES, num_swdge_queues=4)

    # ---- external inputs ----
    zT_in = nc.dram_tensor("zT", [d_in, nt * P], f32, kind="ExternalInput")
    ewbd_in = nc.dram_tensor("ewbd", [P, nt * degcol], f32,
                             kind="ExternalInput")
    ewT_in = nc.dram_tensor("ewT", [P, totch], f32, kind="ExternalInput")
    dmT_in = nc.dram_tensor("dmT", [P, totch], f32, kind="ExternalInput")
    ewTb_in = nc.dram_tensor("ewTb", [P, totch], bf16, kind="ExternalInput")
    dmTb_in = nc.dram_tensor("dmTb", [P, totch], bf16, kind="ExternalInput")
    idx_in = nc.dram_tensor("idxL", [P, totch * 8], i16, kind="ExternalInput")
    iotab_in = nc.dram_tensor("iotab", [P, P], bf16, kind="ExternalInput")
    iota_in = nc.dram_tensor("iota", [P, P], f32, kind="ExternalInput")
    rmask_in = nc.dram_tensor("rmask", [P, 1], f32, kind="ExternalInput")
    ident_in = nc.dram_tensor("ident", [P, P], f32, kind="ExternalInput")
    W_in = [nc.dram_tensor(f"W{i}", [a, b], f32, kind="ExternalInput")
            for i, (a, b) in enumerate(dims)]
    b2_in = nc.dram_tensor("b2", [1, d_in], f32, kind="ExternalInput")
    gm_in = [nc.dram_tensor(f"gm{i}", [1, d_hid], f32, kind="ExternalInput")
             for i in range(2)]
    bt_in = [nc.dram_tensor(f"bt{i}", [1, d_hid], f32, kind="ExternalInput")
             for i in range(2)]
    out_t = nc.dram_tensor("out", [nloc, d_in], f32, kind="ExternalOutput")

    with tile.TileContext(nc) as tc:
        with (
            tc.tile_pool(name="sb", bufs=1) as sb,
            tc.tile_pool(name="wk", bufs=1) as wk,
            tc.tile_pool(name="ps", bufs=1, space="PSUM") as ps,
            tc.tile_pool(name="dram", bufs=1, space="DRAM") as dram,
        ):
            # ---- persistent SBUF state ----
            xT = sb.tile([P, nt * P], f32)          # x^T for the next matmul
            xw_sb = sb.tile([P, nt * d_hid], f32)   # local xw rows
            h_sb = sb.tile([P, nt * d_hid], f32)    # partials, then h
            ewT = sb.tile([P, totch], f32)
            dmT = sb.tile([P, totch], f32)
            ewTb = sb.tile([P, totch], bf16)
            dmTb = sb.tile([P, totch], bf16)
            idxs = sb.tile([P, totch * 8], i16)
            iotab = sb.tile([P, P], bf16)
            iota = sb.tile([P, P], f32)
            rmask = sb.tile([P, 1], f32)
            ident = sb.tile([P, P], f32)
            Ws = [sb.tile([dims[i][0], dims[i][1]], f32, name=f"Wt{i}")
                  for i in range(3)]
            b2r = sb.tile([1, d_in], f32)
            gmr = [sb.tile([1, d_hid], f32, name=f"gmt{i}") for i in range(2)]
            btr = [sb.tile([1, d_hid], f32, name=f"btt{i}") for i in range(2)]
            ones_col = sb.tile([P, 1], f32)
            ones_row = sb.tile([1, P], f32)
            deg = sb.tile([P, nt], f32)
            dinv = sb.tile([P, nt], f32)
            dinv2 = sb.tile([P, nt], f32)
            b2bc = sb.tile([P, d_in], f32)
            statrow = sb.tile([1, 2 * d_hid], f32)
            srow = sb.tile([1, d_hid], f32)
            brow = sb.tile([1, d_hid], f32)
            scol = sb.tile([P, 1], f32)
            bcol = sb.tile([P, 1], f32)

            nc.sync.dma_start(xT[:dims[0][0], :], zT_in[:])
            nc.sync.dma_start(ewT[:], ewT_in[:])
            nc.sync.dma_start(dmT[:], dmT_in[:])
            nc.sync.dma_start(ewTb[:], ewTb_in[:])
            nc.sync.dma_start(dmTb[:], dmTb_in[:])
            nc.sync.dma_start(idxs[:], idx_in[:])
            nc.sync.dma_start(iotab[:], iotab_in[:])
            nc.sync.dma_start(iota[:], iota_in[:])
            nc.sync.dma_start(rmask[:], rmask_in[:])
            nc.sync.dma_start(ident[:], ident_in[:])
            for i in range(3):
                nc.sync.dma_start(Ws[i][:], W_in[i][:])
            nc.sync.dma_start(b2r[:], b2_in[:])
            for i in range(2):
                nc.sync.dma_start(gmr[i][:], gm_in[i][:])
                nc.sync.dma_start(btr[i][:], bt_in[i][:])
            nc.vector.memset(ones_col[:], 1.0)
            nc.vector.memset(ones_row[:], 1.0)

            # ---- one-shot timing probes (dummy data, results unused) ----
            if os.environ.get("PROBE", "") == "1":
                ptab = dram.tile([2048, 64], f32, name="ptab")
                pidx16 = sb.tile([P, 64], i16, name="pidx16")
                pidx32 = sb.tile([P, 1], mybir.dt.int32, name="pidx32")
                nc.vector.memset(pidx16[:], 0.0)
                nc.vector.memset(pidx32[:], 0.0)
                for rep in range(8):
                    pg1 = wk.tile([P, 8 * 64], f32, tag="pg1", bufs=1,
                                  name="pg1")
                    nc.gpsimd.dma_gather(
                        pg1[:].rearrange("p (c d) -> p c d", c=8),
                        ptab[:], pidx16[:], 1024, 1024, 64)
                for rep in range(8):
                    pg2 = wk.tile([P, 8 * 64], f32, tag="pg2", bufs=1,
                                  name="pg2")
                    nc.gpsimd.dma_gather(
                        pg2[:].rearrange("p (c d) -> p c d", c=8),
                        ptab[:], pidx16[:], 1024, 1024, 64,
                        single_packet=False)
                for rep in range(8):
                    pg3 = wk.tile([P, 64], f32, tag="pg3", bufs=1,
                                  name="pg3")
                    nc.gpsimd.indirect_dma_start(
                        out=pg3[:], out_offset=None, in_=ptab[:],
                        in_offset=bass.IndirectOffsetOnAxis(
                            ap=pidx32[:, :1], axis=0))
                pidx32b = sb.tile([P, 32], mybir.dt.int32, name="pidx32b")
                nc.vector.memset(pidx32b[:], 0.0)
                for rep in range(6):
                    pg4 = wk.tile([P, 8, 64], f32, tag="pg4", bufs=1,
                                  name="pg4")
                    nc.gpsimd.indirect_dma_start(
                        out=pg4[:], out_offset=None, in_=ptab[:],
                        in_offset=bass.IndirectOffsetOnAxis(
                            ap=pidx32b[:, :8], axis=0))
                for rep in range(6):
                    pg5 = wk.tile([P, 32, 64], f32, tag="pg5", bufs=1,
                                  name="pg5")
                    nc.gpsimd.indirect_dma_start(
                        out=pg5[:], out_offset=None, in_=ptab[:],
                        in_offset=bass.IndirectOffsetOnAxis(
                            ap=pidx32b[:, :32], axis=0))
                ptab2 = sb.tile([P, 2048], mybir.dt.uint32, name="ptab2")
                pidx16b = sb.tile([P, 64], i16, name="pidx16b")
                nc.vector.memset(ptab2[:], 0.0)
                nc.vector.memset(pidx16b[:], 0.0)
                for rep in range(6):
                    pg6 = wk.tile([P, 1024], mybir.dt.uint32, tag="pg6",
                                  bufs=1, name="pg6")
                    nc.gpsimd.ap_gather(
                        pg6[:].rearrange("p (n d) -> p n d", d=1),
                        ptab2[:].rearrange("p (n d) -> p n d", d=1),
                        pidx16b[:], channels=128, num_elems=2048, d=1,
                        num_idxs=1024)

            # ---- degree -> dinv, dinv2 ----
            ewbd = wk.tile([P, nt * degcol], f32)
            nc.sync.dma_start(ewbd[:], ewbd_in[:])
            nc.vector.tensor_reduce(
                out=deg[:], in_=ewbd[:].rearrange("p (t j) -> p t j", t=nt),
                axis=mybir.AxisListType.X, op=ALU.add)
            nc.vector.tensor_scalar(out=deg[:], in0=deg[:], scalar1=1.0,
                                    scalar2=None, op0=ALU.add)
            sqd = wk.tile([P, nt], f32)
            nc.scalar.activation(sqd[:], deg[:], ACTF.Sqrt)
            nc.vector.reciprocal(dinv[:], sqd[:])
            nc.vector.tensor_tensor(out=dinv2[:], in0=dinv[:], in1=dinv[:],
                                    op=ALU.mult)

            # debug truncation: KSTOP="<nlayers>,<stage>"
            kstop = os.environ.get("KSTOP", "")
            if kstop:
                nlayers_dbg, stage_dbg = (int(x) for x in kstop.split(","))
            else:
                nlayers_dbg, stage_dbg = 3, 99

            # broadcast b2 across partitions (PE trick)
            bc_ps = ps.tile([P, d_hid], f32, tag="statA")
            nc.tensor.matmul(out=bc_ps[:, :d_in], lhsT=ones_row[:],
                             rhs=b2r[:], start=True, stop=True)
            nc.scalar.copy(b2bc[:], bc_ps[:, :d_in])

            for layer in range(3):
                if layer > nlayers_dbg:
                    break
                part_layer = layer == nlayers_dbg
                din, dout = dims[layer]
                # bf16 message path where the 256B DMA-elem floor allows
                mdt = bf16 if (USE_BF16 and dout * 2 % 256 == 0) else f32
                W = Ws[layer]

                # ---- local xw, y rows (split into sub-range buffers) ----
                ag_in = [dram.tile([qsizes[q], dout], mdt, tag=f"agin{q}",
                                   name=f"ag_in{q}") for q in range(SPLITS)]
                for t in range(nt):
                    xw_ps = ps.tile([P, dout], f32, tag="xwps", bufs=2,
                                    name="xw_ps")
                    nc.tensor.matmul(out=xw_ps[:],
                                     lhsT=xT[:din, t * P:(t + 1) * P],
                                     rhs=W[:], start=True, stop=True)
                    nc.scalar.copy(xw_sb[:, t * dout:(t + 1) * dout],
                                   xw_ps[:])
                    y_t = wk.tile([P, dout], mdt, tag="y", bufs=3, name="y_t")
                    nc.vector.tensor_scalar(out=y_t[:], in0=xw_ps[:],
                                            scalar1=dinv[:, t:t + 1],
                                            scalar2=None, op0=ALU.mult)
                    for (a, b, q, off) in ysegs[t]:
                        nc.sync.dma_start(ag_in[q][off:off + (b - a), :],
                                          y_t[a:b, :])
                if part_layer and stage_dbg < 1:
                    break
                y_full = [dram.tile([NCORES * qsizes[q], dout], mdt,
                                    tag=f"yfull{q}", name=f"y_full{q}",
                                    addr_space="Shared")
                          for q in range(SPLITS)]
                if not no_collectives:
                    for q in range(SPLITS):
                        nc.gpsimd.collective_compute(
                            "AllGather", ALU.bypass,
                            replica_groups=[list(range(NCORES))],
                            ins=[ag_in[q][:].opt()],
                            outs=[y_full[q][:].opt()])

                # ---- gather calls ----
                if part_layer and stage_dbg < 2:
                    break
                msg_tiles = []
                for ci, (ck0, ncnk, q) in enumerate(calls):
                    mt = wk.tile([P, CALLC * dout], mdt, tag="msg",
                                 bufs=MSG_BUFS, name="mt")
                    nidx = ncnk * P
                    nc.gpsimd.dma_gather(
                        mt[:, :ncnk * dout].rearrange("p (c d) -> p c d",
                                                      c=ncnk),
                        y_full[q][:], idxs[:, ck0 * 8:(ck0 + ncnk) * 8],
                        nidx, nidx, dout, queue_num=ci % 4)
                    msg_tiles.append(mt)

                def msg_slice(gk):
                    ci, off = chunk_call[gk]
                    return msg_tiles[ci][:, off * dout:(off + 1) * dout]

                def do_chunks(t, q, agg_ps):
                    lst = cot[(t, q)]
                    n = len(lst)
                    gk0 = lst[0]
                    assert lst == list(range(gk0, gk0 + n))
                    # bulk one-hot build: S[p,(g,m)] = ew[p,g]*(dm[p,g]==m)
                    dmv = (dmTb if mdt == bf16 else dmT)[:, gk0:gk0 + n]
                    ewv = (ewTb if mdt == bf16 else ewT)[:, gk0:gk0 + n]
                    io = iotab if mdt == bf16 else iota
                    Sg = wk.tile([P, n * P], mdt, tag="Sg", bufs=10,
                                 name="Sg")
                    Sg3 = Sg[:].rearrange("p (g m) -> p g m", g=n)
                    nc.vector.tensor_tensor(
                        out=Sg3,
                        in0=dmv.unsqueeze(2).broadcast_to([P, n, P]),
                        in1=io[:].unsqueeze(1).broadcast_to([P, n, P]),
                        op=ALU.is_equal)
                    nc.vector.tensor_tensor(
                        out=Sg3, in0=Sg3,
                        in1=ewv.unsqueeze(2).broadcast_to([P, n, P]),
                        op=ALU.mult)
                    for j, gk in enumerate(lst):
                        nc.tensor.matmul(out=agg_ps[:],
                                         lhsT=Sg[:, j * P:(j + 1) * P],
                                         rhs=msg_slice(gk),
                                         start=(j == 0),
                                         stop=(j == len(lst) - 1))

                # ---- aggregation phases (q-major; partials in h_sb) ----
                if part_layer and stage_dbg < 3:
                    break
                has_part = [False] * nt
                stA = stB = None
                for q in range(SPLITS):
                    lastq = q == SPLITS - 1
                    if layer < 2 and lastq:
                        stA = ps.tile([1, d_hid], f32, tag="statA",
                                      name="stA")
                        stB = ps.tile([1, d_hid], f32, tag="statB",
                                      name="stB")
                    for t in range(nt):
                        hs = h_sb[:, t * dout:(t + 1) * dout]
                        have = bool(cot[(t, q)])
                        agg_ps = None
                        if have:
                            agg_ps = ps.tile([P, dout], f32, tag="aggps",
                                             bufs=2, name="agg_ps")
                            do_chunks(t, q, agg_ps)
                            if has_part[t]:
                                nc.vector.tensor_tensor(out=hs, in0=agg_ps[:],
                                                        in1=hs, op=ALU.add)
                            elif not lastq:
                                nc.scalar.copy(hs, agg_ps[:])
                                has_part[t] = True
                            # lastq && no partial: fold below from PSUM
                        if not lastq:
                            continue
                        # ---- per-tile post: h = dinv*agg + dinv2*xw ----
                        xs = xw_sb[:, t * dout:(t + 1) * dout]
                        wt = wk.tile([P, dout], f32, tag="wsl", bufs=2,
                                     name="wt")
                        nc.vector.tensor_scalar(out=wt[:], in0=xs,
                                                scalar1=dinv2[:, t:t + 1],
                                                scalar2=None, op0=ALU.mult)
                        if have and not has_part[t]:
                            nc.vector.tensor_scalar(out=hs, in0=agg_ps[:],
                                                    scalar1=dinv[:, t:t + 1],
                                                    scalar2=None,
                                                    op0=ALU.mult)
                        elif has_part[t]:
                            nc.vector.tensor_scalar(out=hs, in0=hs,
                                                    scalar1=dinv[:, t:t + 1],
                                                    scalar2=None,
                                                    op0=ALU.mult)
                        else:
                            nc.vector.memset(hs, 0.0)
                        nc.vector.tensor_tensor(out=hs, in0=hs, in1=wt[:],
                                                op=ALU.add)
                        if layer < 2:
                            if t == nt - 1 and last_rows < P:
                                nc.vector.tensor_scalar(
                                    out=hs, in0=hs, scalar1=rmask[:],
                                    scalar2=None, op0=ALU.mult)
                            nc.tensor.matmul(out=stA[:, :dout],
                                             lhsT=ones_col[:], rhs=hs,
                                             start=(t == 0),
                                             stop=(t == nt - 1))
                            sq = wk.tile([P, dout], f32, tag="sq", bufs=2,
                                         name="sq")
                            nc.scalar.activation(sq[:], hs, ACTF.Square)
                            nc.tensor.matmul(out=stB[:, :dout],
                                             lhsT=ones_col[:], rhs=sq[:],
                                             start=(t == 0),
                                             stop=(t == nt - 1))
                        else:
                            rows = last_rows if t == nt - 1 else P
                            o_t = wk.tile([P, dout], f32, tag="y", bufs=3,
                                          name="o_t")
                            nc.vector.tensor_tensor(out=o_t[:], in0=hs,
                                                    in1=b2bc[:], op=ALU.add)
                            nc.sync.dma_start(out_t[t * P:t * P + rows, :],
                                              o_t[:rows, :])

                if part_layer and stage_dbg < 4:
                    break
                if layer < 2:
                    # ---- BN stats AllReduce -> scale/shift columns ----
                    nc.scalar.copy(statrow[:, :dout], stA[:, :dout])
                    nc.scalar.copy(statrow[:, dout:2 * dout], stB[:, :dout])
                    st_in = dram.tile([1, 2 * d_hid], f32, tag="stin",
                                      name="st_in")
                    st_out = dram.tile([1, 2 * d_hid], f32, tag="stout",
                                       name="st_out", addr_space="Shared")
                    nc.sync.dma_start(st_in[:], statrow[:])
                    if not no_collectives:
                        nc.gpsimd.collective_compute(
                            "AllReduce", ALU.add,
                            replica_groups=[list(range(NCORES))],
                            ins=[st_in[:].opt()], outs=[st_out[:].opt()])
                    nc.sync.dma_start(statrow[:], st_out[:])
                    mrow = wk.tile([1, dout], f32, tag="mrow", name="mrow")
                    vrow = wk.tile([1, dout], f32, tag="vrow", name="vrow")
                    nc.vector.tensor_scalar(out=mrow[:], in0=statrow[:, :dout],
                                            scalar1=1.0 / n_nodes,
                                            scalar2=None, op0=ALU.mult)
                    nc.vector.tensor_scalar(out=vrow[:],
                                            in0=statrow[:, dout:2 * dout],
                                            scalar1=1.0 / n_nodes,
                                            scalar2=None, op0=ALU.mult)
                    m2 = wk.tile([1, dout], f32, tag="m2", name="m2")
                    nc.vector.tensor_tensor(out=m2[:], in0=mrow[:],
                                            in1=mrow[:], op=ALU.mult)
                    nc.vector.tensor_tensor(out=vrow[:], in0=vrow[:],
                                            in1=m2[:], op=ALU.subtract)
                    nc.vector.tensor_scalar(out=vrow[:], in0=vrow[:],
                                            scalar1=BN_EPS, scalar2=None,
                                            op0=ALU.add)
                    nc.scalar.activation(m2[:], vrow[:], ACTF.Sqrt)
                    nc.vector.reciprocal(vrow[:], m2[:])
                    nc.vector.tensor_tensor(out=srow[:, :dout], in0=vrow[:],
                                            in1=gmr[layer][:, :dout],
                                            op=ALU.mult)
                    nc.vector.tensor_tensor(out=m2[:], in0=srow[:, :dout],
                                            in1=mrow[:], op=ALU.mult)
                    nc.vector.tensor_tensor(out=brow[:, :dout],
                                            in0=btr[layer][:, :dout],
                                            in1=m2[:], op=ALU.subtract)
                    # transpose scale/shift rows into per-partition columns
                    tc1 = ps.tile([P, 1], f32, tag="statA", name="tc1")
                    nc.tensor.transpose(out=tc1[:dout, :],
                                        in_=srow[:, :dout],
                                        identity=ident[:1, :1])
                    nc.scalar.copy(scol[:dout, :], tc1[:dout, :])
                    tc2 = ps.tile([P, 1], f32, tag="statB", name="tc2")
                    nc.tensor.transpose(out=tc2[:dout, :],
                                        in_=brow[:, :dout],
                                        identity=ident[:1, :1])
                    nc.scalar.copy(bcol[:dout, :], tc2[:dout, :])

                    # ---- x = relu(s*h + b) fused on ACT in T layout ----
                    for t in range(nt):
                        hs = h_sb[:, t * dout:(t + 1) * dout]
                        tp = ps.tile([P, P], f32, tag="tpps", bufs=2,
                                     name="tp")
                        nc.tensor.transpose(out=tp[:dout, :], in_=hs,
                                            identity=ident[:])
                        nc.scalar.activation(xT[:dout, t * P:(t + 1) * P],
                                             tp[:dout, :], ACTF.Relu,
                                             bias=bcol[:dout, :],
                                             scale=scol[:dout, :])
    nc.compile()
    return nc


def prepare(z_nodes, src, dst, edge_weight,
            W0, b0, W1, b1, W2, b2,
            gamma0, beta0, gamma1, beta1):
    """Host-side restructuring + program build; returns (nc, in_maps)."""
    z = np.asarray(z_nodes, np.float32)
    src = np.asarray(src).astype(np.int64)
    dst = np.asarray(dst).astype(np.int64)
    ew = np.asarray(edge_weight, np.float32)
    n_nodes, d_in = z.shape
    d_hid = np.asarray(W0).shape[1]
    assert n_nodes % NCORES == 0

    perm = None
    if math.ceil((n_nodes // NCORES) / P) >= 4:
        perm = _balance_perm(src, dst, n_nodes)
        inv = np.empty_like(perm)
        inv[perm] = np.arange(n_nodes)
        z = z[inv]
        src = perm[src]
        dst = perm[dst]

    shared, per_core = _edge_structure(src, dst, ew, n_nodes)
    nloc, nt = shared["nloc"], shared["ntiles"]

    nc = _build_program(n_nodes, d_in, d_hid, shared)

    iota = np.tile(np.arange(P, dtype=np.float32), (P, 1))
    rmask = np.zeros((P, 1), np.float32)
    nlr = nloc - (nt - 1) * P
    rmask[:nlr] = 1.0
    consts = {
        "rmask": rmask,
        "iotab": _to_bf16(iota),
        "iota": np.ascontiguousarray(iota),
        "ident": np.eye(P, dtype=np.float32),
        "W0": np.asarray(W0, np.float32), "W1": np.asarray(W1, np.float32),
        "W2": np.asarray(W2, np.float32),
        "b2": np.asarray(b2, np.float32).reshape(1, -1),
        "gm0": np.asarray(gamma0, np.float32).reshape(1, -1),
        "gm1": np.asarray(gamma1, np.float32).reshape(1, -1),
        "bt0": np.asarray(beta0, np.float32).reshape(1, -1),
        "bt1": np.asarray(beta1, np.float32).reshape(1, -1),
    }
    in_maps = []
    for c in range(NCORES):
        pc = per_core[c]
        zc = z[c * nloc:(c + 1) * nloc]
        zT = np.zeros((d_in, nt * P), np.float32)
        zT[:, :nloc] = zc.T
        in_maps.append({**consts, "zT": zT, "ewbd": pc["ewbd"],
                        "ewT": pc["ewT"], "dmT": pc["dmT"],
                        "ewTb": _to_bf16(pc["ewT"]),
                        "dmTb": _to_bf16(pc["dmT"]),
                        "idxL": pc["idxL"]})
    return nc, in_maps, perm


def kernel(**inputs):
    global LAST_RESULTS
    nc, in_maps, perm = prepare(**inputs)
    res = run_bass_kernel_spmd(nc, in_maps, core_ids=list(range(NCORES)))
    LAST_RESULTS = res
    out = np.concatenate([res.results[c]["out"] for c in range(NCORES)], 0)
    if perm is not None:
        out = out[perm]
    return out



# revision 46
# speedup vs baseline: 1.0158x; 1.0158x over previous
"""Distributed GCN (3-layer CNF dynamics GNN) on 8 Trainium2 NeuronCores.

Math (per reference):
    gcn(x) = D^-1/2 (A + I) D^-1/2 (x W) + b  with self-loop weight 1
    h0 = relu(bn(gcn0(z)));  h1 = relu(bn(gcn1(h0)));  out = gcn2(h1)

Sharding: nodes are split contiguously across the 8 cores (6250 each); edges
are owned by the dst core.  Per layer, each core computes xw for its local
nodes, scales rows by dinv (folds the src-side normalization) and all-gathers
the scaled rows so every core holds the full message table y in HBM.  The
all-gather is split into SPLITS sub-gathers over node sub-ranges (block-cyclic
tables, gather indices remapped host-side) so edge processing on sub-range 0
overlaps the remaining sub-gathers.  Each core gathers y[src] rows for its
edges via SWDGE dma_gather (<=1024 descriptors per call: hard ring limit) and
reduces them per dst-node tile with a one-hot selection-matrix matmul on the
PE: for a chunk of 128 edges, S[e, m] = ew[e] * (dst_local[e] == m), and PSUM
accumulates  agg[m, :] += S^T @ msgs.  Layers 0/1 run the message path in
bf16 (y, S, gathered msgs; f32 PSUM accumulate); layer 2 stays f32 (the
256B-per-descriptor DMA floor forbids 64-wide bf16 rows).  The dst-side
normalization, self-loop term and batchnorm (stats via ones-matmul +
AllReduce; apply fused into one scalar-engine relu-affine in transposed
layout) follow per node tile.

All edge bookkeeping (chunk grid, padding, gather index layout) is pure
integer restructuring done host-side in numpy; all float math is on device.
"""

import math
import os

import numpy as np

import concourse.bacc as bacc
import concourse.bass as bass
import concourse.mybir as mybir
import concourse.tile as tile
from concourse.bass_utils import run_bass_kernel_spmd

P = 128
NCORES = 8
SPLITS = 2              # all-gather split (node sub-ranges per rank)
CALLC = 8               # 128-edge chunks per dma_gather call (<=1024 descs)
MSG_BUFS = 8
S_BUFS = 24
BN_EPS = 1e-5

LAST_RESULTS = None     # test harness peeks exec_time_ns here

f32 = mybir.dt.float32
bf16 = mybir.dt.bfloat16
i16 = mybir.dt.int16
ALU = mybir.AluOpType
ACTF = mybir.ActivationFunctionType
USE_BF16 = os.environ.get("NOBF16", "") != "1"


def _to_bf16(a):
    import ml_dtypes
    return np.asarray(a, dtype=ml_dtypes.bfloat16)


def _edge_structure(src, dst, ew, n_nodes):
    """Host-side integer restructuring: per-core padded edge streams.

    Nodes are range-partitioned over cores; each core's local node range is
    further cut into SPLITS sub-ranges.  The gather table for sub-range q is
    the block-cyclic concat of every rank's q-th sub-range, so gather index
    = rank * qsize[q] + (local - qoff[q])  (always < NCORES*qsize => int16).

    Returns (shared, per_core): `shared` is the chunk grid (identical across
    cores — one SPMD program), `per_core` the padded data arrays.
    """
    nloc = n_nodes // NCORES
    ntiles = math.ceil(nloc / P)
    if ntiles >= 4:
        # tile-aligned sub-ranges so no tile straddles a gather sub-table
        qt0 = (ntiles + 1) // 2
        qsizes = [qt0 * P, nloc - qt0 * P]
    else:
        qsizes = [len(a) for a in np.array_split(np.arange(nloc), SPLITS)]
    qoff = np.concatenate([[0], np.cumsum(qsizes)])  # [SPLITS+1]
    assert NCORES * max(qsizes) < 32768, "gather index must fit int16"
    core_of = dst // nloc

    percore_raw = []
    counts = np.zeros((NCORES, ntiles, SPLITS), np.int64)
    for c in range(NCORES):
        m = core_of == c
        s_c = src[m]
        d_c = dst[m] - c * nloc
        w_c = ew[m]
        t_c = d_c // P
        sr = s_c // nloc                       # src owner rank
        sl = s_c % nloc                        # src local row
        q_c = np.searchsorted(qoff[1:-1], sl, side="right")
        gidx = sr * np.asarray(qsizes)[q_c] + (sl - qoff[q_c])
        np.add.at(counts[c], (t_c, q_c), 1)
        percore_raw.append((gidx, d_c, w_c, t_c, q_c))

    K = np.ceil(counts / P).astype(np.int64).max(axis=0)  # [ntiles, SPLITS]
    totch = int(K.sum())

    # stream order: q-major, tiles ascending within q
    chunk_of_bucket = {}
    gk = 0
    stream = []
    for q in range(SPLITS):
        for t in range(ntiles):
            chunk_of_bucket[(t, q)] = gk
            for _ in range(int(K[t, q])):
                stream.append((t, q))
                gk += 1
    assert gk == totch

    # dma_gather calls: consecutive chunks of one sub-range, up to CALLC each
    calls = []
    gk = 0
    for q in range(SPLITS):
        nchunks_q = int(K[:, q].sum())
        done = 0
        while done < nchunks_q:
            n = min(CALLC, nchunks_q - done)
            calls.append((gk, n, q))
            gk += n
            done += n
    chunk_call = {}
    for ci, (ck0, n, _q) in enumerate(calls):
        for j in range(n):
            chunk_call[ck0 + j] = (ci, j)

    per_core = []
    maxdeg = 0
    for c in range(NCORES):
        gidx, d_c, w_c, t_c, q_c = percore_raw[c]
        ew_seq = np.zeros(totch * P, np.float32)
        dm_seq = np.zeros(totch * P, np.float32)
        ix_seq = np.zeros(totch * P, np.int16)
        order = np.lexsort((t_c, q_c))
        g_o, d_o, w_o = gidx[order], d_c[order], w_c[order]
        pos = 0
        for q in range(SPLITS):
            for t in range(ntiles):
                n = int(counts[c, t, q])
                if n == 0:
                    continue
                slc = slice(pos, pos + n)
                base = chunk_of_bucket[(t, q)] * P
                ew_seq[base:base + n] = w_o[slc]
                dm_seq[base:base + n] = (d_o[slc] % P).astype(np.float32)
                ix_seq[base:base + n] = g_o[slc].astype(np.int16)
                pos += n
        assert pos == len(d_c)
        ewT = np.ascontiguousarray(ew_seq.reshape(totch, P).T)
        dmT = np.ascontiguousarray(dm_seq.reshape(totch, P).T)
        idx16 = np.ascontiguousarray(ix_seq.reshape(totch * 8, 16).T)
        idxL = np.tile(idx16, (8, 1))

        degs = np.bincount(d_c, minlength=ntiles * P)
        maxdeg = max(maxdeg, int(degs.max(initial=0)))
        per_core.append({"ewT": ewT, "dmT": dmT, "idxL": idxL, "d_c": d_c,
                         "w_c": w_c})

    degcol = max(8, math.ceil(maxdeg / 8) * 8)
    for c in range(NCORES):
        pc = per_core[c]
        d_c, w_c = pc.pop("d_c"), pc.pop("w_c")
        ewbd = np.zeros((ntiles * P, degcol), np.float32)
        if len(d_c):
            o = np.argsort(d_c, kind="stable")
            ds, ws = d_c[o], w_c[o]
            starts = np.zeros(ntiles * P + 1, np.int64)
            np.cumsum(np.bincount(ds, minlength=ntiles * P), out=starts[1:])
            slot = np.arange(len(ds)) - starts[ds]
            ewbd[ds, slot] = ws
        pc["ewbd"] = np.ascontiguousarray(
            ewbd.reshape(ntiles, P, degcol).transpose(1, 0, 2)
            .reshape(P, ntiles * degcol))

    chunks_of_tile = {(t, q): [] for t in range(ntiles) for q in range(SPLITS)}
    for gk, (t, q) in enumerate(stream):
        chunks_of_tile[(t, q)].append(gk)

    # per-tile y-row DMA segments: (row_lo, row_hi, q, offset_in_agin_q)
    ysegs = []
    for t in range(ntiles):
        r0 = t * P
        rows = min(P, nloc - r0)
        segs = []
        for q in range(SPLITS):
            a = max(r0, int(qoff[q]))
            b = min(r0 + rows, int(qoff[q + 1]))
            if a < b:
                segs.append((a - r0, b - r0, q, a - int(qoff[q])))
        ysegs.append(segs)

    shared = {"nloc": nloc, "ntiles": ntiles, "totch": totch, "K": K,
              "calls": calls, "chunk_call": chunk_call,
              "chunks_of_tile": chunks_of_tile, "degcol": degcol,
              "qsizes": qsizes, "ysegs": ysegs}
    return shared, per_core


def _balance_perm(src, dst, n_nodes):
    """Within-core node permutation equalizing per-tile in-edge counts.

    Best-fit-decreasing: pack nodes into tiles so every regular tile's
    in-edge total is <= 16*P (=> 16 chunks, no ceil waste); the remainder
    concentrates in tile 0 (same index on every core, so the max-over-cores
    chunk grid stays tight).  Returns perm with perm[orig_id] = new_id;
    node stays on its original core, only the local row changes.
    """
    nloc = n_nodes // NCORES
    nt = math.ceil(nloc / P)
    qt0 = (nt + 1) // 2
    perm = np.empty(n_nodes, np.int64)
    ecap = 8 * P                     # per-(tile, src-half) chunk budget
    # provisional src-half of every node (kept exact: packing never moves a
    # node across the half boundary)
    half = (np.arange(n_nodes) % nloc) >= qt0 * P
    for c in range(NCORES):
        lo, hi = c * nloc, (c + 1) * nloc
        m = (dst >= lo) & (dst < hi)
        d_loc = dst[m] - lo
        # per-node in-degree split by src half
        sh_edges = half[src[m]]
        d0 = np.bincount(d_loc[~sh_edges], minlength=nloc)
        d1 = np.bincount(d_loc[sh_edges], minlength=nloc)
        newloc = np.empty(nloc, np.int64)
        for hsel, t_lo, t_hi, ovf in ((False, 0, qt0, 0),
                                      (True, qt0, nt, qt0)):
            ids = np.where(half[lo:hi] == hsel)[0]
            dg0, dg1 = d0[ids], d1[ids]
            order = np.argsort(-(dg0 + dg1), kind="stable")
            ntl = t_hi - t_lo
            caps = np.full(ntl, P, np.int64)
            if t_hi == nt:
                caps[-1] = nloc - (nt - 1) * P
            l0 = np.zeros(ntl, np.int64)
            l1 = np.zeros(ntl, np.int64)
            used = np.zeros(ntl, np.int64)
            for j in order:
                a, b = dg0[j], dg1[j]
                free = used < caps
                fit = free & (l0 + a <= ecap) & (l1 + b <= ecap)
                fit[ovf - t_lo] = False
                if fit.any():
                    t = int(np.argmax(np.where(fit, l0 + l1, -1)))
                elif free[ovf - t_lo]:
                    t = ovf - t_lo
                else:
                    t = int(np.argmax(np.where(free, -(l0 + l1),
                                               -(1 << 40))))
                newloc[ids[j]] = (t_lo + t) * P + used[t]
                used[t] += 1
                l0[t] += a
                l1[t] += b
        perm[lo:hi] = lo + newloc
    return perm


def _build_program(n_nodes, d_in, d_hid, shared, no_collectives=False):
    """Emit the SPMD Bass program (same for every core)."""
    nloc = shared["nloc"]
    nt = shared["ntiles"]
    totch = shared["totch"]
    calls = shared["calls"]
    chunk_call = shared["chunk_call"]
    cot = shared["chunks_of_tile"]
    degcol = shared["degcol"]
    qsizes = shared["qsizes"]
    ysegs = shared["ysegs"]
    last_rows = nloc - (nt - 1) * P
    dims = [(d_in, d_hid), (d_hid, d_hid), (d_hid, d_in)]

    nc = bacc.Bacc("TRN2", target_bir_lowering=False, debug=False,
                   num_devices=NCORES, num_swdge_queues=4)

    # ---- external inputs ----
    zT_in = nc.dram_tensor("zT", [d_in, nt * P], f32, kind="ExternalInput")
    ewbd_in = nc.dram_tensor("ewbd", [P, nt * degcol], f32,
                             kind="ExternalInput")
    ewT_in = nc.dram_tensor("ewT", [P, totch], f32, kind="ExternalInput")
    dmT_in = nc.dram_tensor("dmT", [P, totch], f32, kind="ExternalInput")
    ewTb_in = nc.dram_tensor("ewTb", [P, totch], bf16, kind="ExternalInput")
    dmTb_in = nc.dram_tensor("dmTb", [P, totch], bf16, kind="ExternalInput")
    idx_in = nc.dram_tensor("idxL", [P, totch * 8], i16, kind="ExternalInput")
    iotab_in = nc.dram_tensor("iotab", [P, P], bf16, kind="ExternalInput")
    iota_in = nc.dram_tensor("iota", [P, P], f32, kind="ExternalInput")
    rmask_in = nc.dram_tensor("rmask", [P, 1], f32, kind="ExternalInput")
    ident_in = nc.dram_tensor("ident", [P, P], f32, kind="ExternalInput")
    W_in = [nc.dram_tensor(f"W{i}", [a, b], f32, kind="ExternalInput")
            for i, (a, b) in enumerate(dims)]
    b2_in = nc.dram_tensor("b2", [1, d_in], f32, kind="ExternalInput")
    gm_in = [nc.dram_tensor(f"gm{i}", [1, d_hid], f32, kind="ExternalInput")
             for i in range(2)]
    bt_in = [nc.dram_tensor(f"bt{i}", [1, d_hid], f32, kind="ExternalInput")
             for i in range(2)]
    out_t = nc.dram_tensor("out", [nloc, d_in], f32, kind="ExternalOutput")

    with tile.TileContext(nc) as tc:
        with (
            tc.tile_pool(name="sb", bufs=1) as sb,
            tc.tile_pool(name="wk", bufs=1) as wk,
            tc.tile_pool(name="ps", bufs=1, space="PSUM") as ps,
            tc.tile_pool(name="dram", bufs=1, space="DRAM") as dram,
        ):
            # ---- persistent SBUF state ----
            xT = sb.tile([P, nt * P], f32)          # x^T for the next matmul
            xw_sb = sb.tile([P, nt * d_hid], f32)   # local xw rows
            h_sb = sb.tile([P, nt * d_hid], f32)    # partials, then h
            ewT = sb.tile([P, totch], f32)
            dmT = sb.tile([P, totch], f32)
            ewTb = sb.tile([P, totch], bf16)
            dmTb = sb.tile([P, totch], bf16)
            idxs = sb.tile([P, totch * 8], i16)
            iotab = sb.tile([P, P], bf16)
            iota = sb.tile([P, P], f32)
            rmask = sb.tile([P, 1], f32)
            ident = sb.tile([P, P], f32)
            Ws = [sb.tile([dims[i][0], dims[i][1]], f32, name=f"Wt{i}")
                  for i in range(3)]
            b2r = sb.tile([1, d_in], f32)
            gmr = [sb.tile([1, d_hid], f32, name=f"gmt{i}") for i in range(2)]
            btr = [sb.tile([1, d_hid], f32, name=f"btt{i}") for i in range(2)]
            ones_col = sb.tile([P, 1], f32)
            ones_row = sb.tile([1, P], f32)
            deg = sb.tile([P, nt], f32)
            dinv = sb.tile([P, nt], f32)
            dinv2 = sb.tile([P, nt], f32)
            b2bc = sb.tile([P, d_in], f32)
            statrow = sb.tile([1, 2 * d_hid], f32)
            srow = sb.tile([1, d_hid], f32)
            brow = sb.tile([1, d_hid], f32)
            scol = sb.tile([P, 1], f32)
            bcol = sb.tile([P, 1], f32)

            nc.sync.dma_start(xT[:dims[0][0], :], zT_in[:])
            nc.sync.dma_start(ewT[:], ewT_in[:])
            nc.sync.dma_start(dmT[:], dmT_in[:])
            nc.sync.dma_start(ewTb[:], ewTb_in[:])
            nc.sync.dma_start(dmTb[:], dmTb_in[:])
            nc.sync.dma_start(idxs[:], idx_in[:])
            nc.sync.dma_start(iotab[:], iotab_in[:])
            nc.sync.dma_start(iota[:], iota_in[:])
            nc.sync.dma_start(rmask[:], rmask_in[:])
            nc.sync.dma_start(ident[:], ident_in[:])
            for i in range(3):
                nc.sync.dma_start(Ws[i][:], W_in[i][:])
            nc.sync.dma_start(b2r[:], b2_in[:])
            for i in range(2):
                nc.sync.dma_start(gmr[i][:], gm_in[i][:])
                nc.sync.dma_start(btr[i][:], bt_in[i][:])
            nc.vector.memset(ones_col[:], 1.0)
            nc.vector.memset(ones_row[:], 1.0)

            # ---- one-shot timing probes (dummy data, results unused) ----
            if os.environ.get("PROBE", "") == "1":
                ptab = dram.tile([2048, 64], f32, name="ptab")
                pidx16 = sb.tile([P, 64], i16, name="pidx16")
                pidx32 = sb.tile([P, 1], mybir.dt.int32, name="pidx32")
                nc.vector.memset(pidx16[:], 0.0)
                nc.vector.memset(pidx32[:], 0.0)
                for rep in range(8):
                    pg1 = wk.tile([P, 8 * 64], f32, tag="pg1", bufs=1,
                                  name="pg1")
                    nc.gpsimd.dma_gather(
                        pg1[:].rearrange("p (c d) -> p c d", c=8),
                        ptab[:], pidx16[:], 1024, 1024, 64)
                for rep in range(8):
                    pg2 = wk.tile([P, 8 * 64], f32, tag="pg2", bufs=1,
                                  name="pg2")
                    nc.gpsimd.dma_gather(
                        pg2[:].rearrange("p (c d) -> p c d", c=8),
                        ptab[:], pidx16[:], 1024, 1024, 64,
                        single_packet=False)
                for rep in range(8):
                    pg3 = wk.tile([P, 64], f32, tag="pg3", bufs=1,
                                  name="pg3")
                    nc.gpsimd.indirect_dma_start(
                        out=pg3[:], out_offset=None, in_=ptab[:],
                        in_offset=bass.IndirectOffsetOnAxis(
                            ap=pidx32[:, :1], axis=0))
                pidx32b = sb.tile([P, 32], mybir.dt.int32, name="pidx32b")
                nc.vector.memset(pidx32b[:], 0.0)
                for rep in range(6):
                    pg4 = wk.tile([P, 8, 64], f32, tag="pg4", bufs=1,
                                  name="pg4")
                    nc.gpsimd.indirect_dma_start(
                        out=pg4[:], out_offset=None, in_=ptab[:],
                        in_offset=bass.IndirectOffsetOnAxis(
                            ap=pidx32b[:, :8], axis=0))
                for rep in range(6):
                    pg5 = wk.tile([P, 32, 64], f32, tag="pg5", bufs=1,
                                  name="pg5")
                    nc.gpsimd.indirect_dma_start(
                        out=pg5[:], out_offset=None, in_=ptab[:],
                        in_offset=bass.IndirectOffsetOnAxis(
                            ap=pidx32b[:, :32], axis=0))
                ptab2 = sb.tile([P, 2048], mybir.dt.uint32, name="ptab2")
                pidx16b = sb.tile([P, 64], i16, name="pidx16b")
                nc.vector.memset(ptab2[:], 0.0)
                nc.vector.memset(pidx16b[:], 0.0)
                for rep in range(6):
                    pg6 = wk.tile([P, 1024], mybir.dt.uint32, tag="pg6",
                                  bufs=1, name="pg6")
                    nc.gpsimd.ap_gather(
                        pg6[:].rearrange("p (n d) -> p n d", d=1),
                        ptab2[:].rearrange("p (n d) -> p n d", d=1),
                        pidx16b[:], channels=128, num_elems=2048, d=1,
                        num_idxs=1024)

            # ---- degree -> dinv, dinv2 ----
            ewbd = wk.tile([P, nt * degcol], f32)
            nc.sync.dma_start(ewbd[:], ewbd_in[:])
            nc.vector.tensor_reduce(
                out=deg[:], in_=ewbd[:].rearrange("p (t j) -> p t j", t=nt),
                axis=mybir.AxisListType.X, op=ALU.add)
            nc.vector.tensor_scalar(out=deg[:], in0=deg[:], scalar1=1.0,
                                    scalar2=None, op0=ALU.add)
            sqd = wk.tile([P, nt], f32)
            nc.scalar.activation(sqd[:], deg[:], ACTF.Sqrt)
            nc.vector.reciprocal(dinv[:], sqd[:])
            nc.vector.tensor_tensor(out=dinv2[:], in0=dinv[:], in1=dinv[:],
                                    op=ALU.mult)

            # debug truncation: KSTOP="<nlayers>,<stage>"
            kstop = os.environ.get("KSTOP", "")
            if kstop:
                nlayers_dbg, stage_dbg = (int(x) for x in kstop.split(","))
            else:
                nlayers_dbg, stage_dbg = 3, 99

            # broadcast b2 across partitions (PE trick)
            bc_ps = ps.tile([P, d_hid], f32, tag="statA")
            nc.tensor.matmul(out=bc_ps[:, :d_in], lhsT=ones_row[:],
                             rhs=b2r[:], start=True, stop=True)
            nc.scalar.copy(b2bc[:], bc_ps[:, :d_in])

            for layer in range(3):
                if layer > nlayers_dbg:
                    break
                part_layer = layer == nlayers_dbg
                din, dout = dims[layer]
                # bf16 message path where the 256B DMA-elem floor allows
                mdt = bf16 if (USE_BF16 and dout * 2 % 256 == 0) else f32
                W = Ws[layer]

                # ---- local xw, y rows (split into sub-range buffers) ----
                ag_in = [dram.tile([qsizes[q], dout], mdt, tag=f"agin{q}",
                                   name=f"ag_in{q}") for q in range(SPLITS)]
                for t in range(nt):
                    xw_ps = ps.tile([P, dout], f32, tag="xwps", bufs=2,
                                    name="xw_ps")
                    nc.tensor.matmul(out=xw_ps[:],
                                     lhsT=xT[:din, t * P:(t + 1) * P],
                                     rhs=W[:], start=True, stop=True)
                    nc.scalar.copy(xw_sb[:, t * dout:(t + 1) * dout],
                                   xw_ps[:])
                    y_t = wk.tile([P, dout], mdt, tag="y", bufs=3, name="y_t")
                    nc.vector.tensor_scalar(out=y_t[:], in0=xw_ps[:],
                                            scalar1=dinv[:, t:t + 1],
                                            scalar2=None, op0=ALU.mult)
                    for (a, b, q, off) in ysegs[t]:
                        nc.sync.dma_start(ag_in[q][off:off + (b - a), :],
                                          y_t[a:b, :])
                if part_layer and stage_dbg < 1:
                    break
                y_full = [dram.tile([NCORES * qsizes[q], dout], mdt,
                                    tag=f"yfull{q}", name=f"y_full{q}",
                                    addr_space="Shared")
                          for q in range(SPLITS)]
                if not no_collectives:
                    for q in range(SPLITS):
                        nc.gpsimd.collective_compute(
                            "AllGather", ALU.bypass,
                            replica_groups=[list(range(NCORES))],
                            ins=[ag_in[q][:].opt()],
                            outs=[y_full[q][:].opt()])

                # ---- gather calls ----
                if part_layer and stage_dbg < 2:
                    break
                msg_tiles = []
                for ci, (ck0, ncnk, q) in enumerate(calls):
                    mt = wk.tile([P, CALLC * dout], mdt, tag="msg",
                                 bufs=MSG_BUFS, name="mt")
                    nidx = ncnk * P
                    nc.gpsimd.dma_gather(
                        mt[:, :ncnk * dout].rearrange("p (c d) -> p c d",
                                                      c=ncnk),
                        y_full[q][:], idxs[:, ck0 * 8:(ck0 + ncnk) * 8],
                        nidx, nidx, dout, queue_num=ci % 4)
                    msg_tiles.append(mt)

                def msg_slice(gk):
                    ci, off = chunk_call[gk]
                    return msg_tiles[ci][:, off * dout:(off + 1) * dout]

                def do_chunks(t, q, agg_ps):
                    lst = cot[(t, q)]
                    n = len(lst)
                    gk0 = lst[0]
                    assert lst == list(range(gk0, gk0 + n))
                    # bulk one-hot build: S[p,(g,m)] = ew[p,g]*(dm[p,g]==m)
                    dmv = (dmTb if mdt == bf16 else dmT)[:, gk0:gk0 + n]
                    ewv = (ewTb if mdt == bf16 else ewT)[:, gk0:gk0 + n]
                    io = iotab if mdt == bf16 else iota
                    Sg = wk.tile([P, n * P], mdt, tag="Sg", bufs=8,
                                 name="Sg")
                    Sg3 = Sg[:].rearrange("p (g m) -> p g m", g=n)
                    nc.vector.tensor_tensor(
                        out=Sg3,
                        in0=dmv.unsqueeze(2).broadcast_to([P, n, P]),
                        in1=io[:].unsqueeze(1).broadcast_to([P, n, P]),
                        op=ALU.is_equal)
                    nc.vector.tensor_tensor(
                        out=Sg3, in0=Sg3,
                        in1=ewv.unsqueeze(2).broadcast_to([P, n, P]),
                        op=ALU.mult)
                    for j, gk in enumerate(lst):
                        nc.tensor.matmul(out=agg_ps[:],
                                         lhsT=Sg[:, j * P:(j + 1) * P],
                                         rhs=msg_slice(gk),
                                         start=(j == 0),
                                         stop=(j == len(lst) - 1))

                # ---- aggregation phases (q-major; partials in h_sb) ----
                if part_layer and stage_dbg < 3:
                    break
                has_part = [False] * nt
                stA = stB = None
                for q in range(SPLITS):
                    lastq = q == SPLITS - 1
                    if layer < 2 and lastq:
                        stA = ps.tile([1, d_hid], f32, tag="statA",
                                      name="stA")
                        stB = ps.tile([1, d_hid], f32, tag="statB",
                                      name="stB")
                    for t in range(nt):
                        hs = h_sb[:, t * dout:(t + 1) * dout]
                        have = bool(cot[(t, q)])
                        agg_ps = None
                        if have:
                            agg_ps = ps.tile([P, dout], f32, tag="aggps",
                                             bufs=2, name="agg_ps")
                            do_chunks(t, q, agg_ps)
                            if has_part[t]:
                                nc.vector.tensor_tensor(out=hs, in0=agg_ps[:],
                                                        in1=hs, op=ALU.add)
                            elif not lastq:
                                nc.scalar.copy(hs, agg_ps[:])
                                has_part[t] = True
                            # lastq && no partial: fold below from PSUM
                        if not lastq:
                            continue
                        # ---- per-tile post: h = dinv*agg + dinv2*xw ----
                        xs = xw_sb[:, t * dout:(t + 1) * dout]
                        wt = wk.tile([P, dout], f32, tag="wsl", bufs=2,
                                     name="wt")
                        nc.vector.tensor_scalar(out=wt[:], in0=xs,
                                                scalar1=dinv2[:, t:t + 1],
                                                scalar2=None, op0=ALU.mult)
                        if have and not has_part[t]:
                            nc.vector.tensor_scalar(out=hs, in0=agg_ps[:],
                                                    scalar1=dinv[:, t:t + 1],
                                                    scalar2=None,
                                                    op0=ALU.mult)
                        elif has_part[t]:
                            nc.vector.tensor_scalar(out=hs, in0=hs,
                                                    scalar1=dinv[:, t:t + 1],
                                                    scalar2=None,
                                                    op0=ALU.mult)
                        else:
                            nc.vector.memset(hs, 0.0)
                        nc.vector.tensor_tensor(out=hs, in0=hs, in1=wt[:],
                                                op=ALU.add)
                        if layer < 2:
                            if t == nt - 1 and last_rows < P:
                                nc.vector.tensor_scalar(
                                    out=hs, in0=hs, scalar1=rmask[:],
                                    scalar2=None, op0=ALU.mult)
                            nc.tensor.matmul(out=stA[:, :dout],
                                             lhsT=ones_col[:], rhs=hs,
                                             start=(t == 0),
                                             stop=(t == nt - 1))
                            sq = wk.tile([P, dout], f32, tag="sq", bufs=2,
                                         name="sq")
                            nc.scalar.activation(sq[:], hs, ACTF.Square)
                            nc.tensor.matmul(out=stB[:, :dout],
                                             lhsT=ones_col[:], rhs=sq[:],
                                             start=(t == 0),
                                             stop=(t == nt - 1))
                        else:
                            rows = last_rows if t == nt - 1 else P
                            o_t = wk.tile([P, dout], f32, tag="y", bufs=3,
                                          name="o_t")
                            nc.vector.tensor_tensor(out=o_t[:], in0=hs,
                                                    in1=b2bc[:], op=ALU.add)
                            nc.sync.dma_start(out_t[t * P:t * P + rows, :],
                                              o_t[:rows, :])

                if part_layer and stage_dbg < 4:
                    break
                if layer < 2:
                    # ---- BN stats AllReduce -> scale/shift columns ----
                    nc.scalar.copy(statrow[:, :dout], stA[:, :dout])
                    nc.scalar.copy(statrow[:, dout:2 * dout], stB[:, :dout])
                    st_in = dram.tile([1, 2 * d_hid], f32, tag="stin",
                                      name="st_in")
                    st_out = dram.tile([1, 2 * d_hid], f32, tag="stout",
                                       name="st_out", addr_space="Shared")
                    nc.sync.dma_start(st_in[:], statrow[:])
                    if not no_collectives:
                        nc.gpsimd.collective_compute(
                            "AllReduce", ALU.add,
                            replica_groups=[list(range(NCORES))],
                            ins=[st_in[:].opt()], outs=[st_out[:].opt()])
                    nc.sync.dma_start(statrow[:], st_out[:])
                    mrow = wk.tile([1, dout], f32, tag="mrow", name="mrow")
                    vrow = wk.tile([1, dout], f32, tag="vrow", name="vrow")
                    nc.vector.tensor_scalar(out=mrow[:], in0=statrow[:, :dout],
                                            scalar1=1.0 / n_nodes,
                                            scalar2=None, op0=ALU.mult)
                    nc.vector.tensor_scalar(out=vrow[:],
                                            in0=statrow[:, dout:2 * dout],
                                            scalar1=1.0 / n_nodes,
                                            scalar2=None, op0=ALU.mult)
                    m2 = wk.tile([1, dout], f32, tag="m2", name="m2")
                    nc.vector.tensor_tensor(out=m2[:], in0=mrow[:],
                                            in1=mrow[:], op=ALU.mult)
                    nc.vector.tensor_tensor(out=vrow[:], in0=vrow[:],
                                            in1=m2[:], op=ALU.subtract)
                    nc.vector.tensor_scalar(out=vrow[:], in0=vrow[:],
                                            scalar1=BN_EPS, scalar2=None,
                                            op0=ALU.add)
                    nc.scalar.activation(m2[:], vrow[:], ACTF.Sqrt)
                    nc.vector.reciprocal(vrow[:], m2[:])
                    nc.vector.tensor_tensor(out=srow[:, :dout], in0=vrow[:],
                                            in1=gmr[layer][:, :dout],
                                            op=ALU.mult)
                    nc.vector.tensor_tensor(out=m2[:], in0=srow[:, :dout],
                                            in1=mrow[:], op=ALU.mult)
                    nc.vector.tensor_tensor(out=brow[:, :dout],
                                            in0=btr[layer][:, :dout],
                                            in1=m2[:], op=ALU.subtract)
                    # transpose scale/shift rows into per-partition columns
                    tc1 = ps.tile([P, 1], f32, tag="statA", name="tc1")
                    nc.tensor.transpose(out=tc1[:dout, :],
                                        in_=srow[:, :dout],
                                        identity=ident[:1, :1])
                    nc.scalar.copy(scol[:dout, :], tc1[:dout, :])
                    tc2 = ps.tile([P, 1], f32, tag="statB", name="tc2")
                    nc.tensor.transpose(out=tc2[:dout, :],
                                        in_=brow[:, :dout],
                                        identity=ident[:1, :1])
                    nc.scalar.copy(bcol[:dout, :], tc2[:dout, :])

                    # ---- x = relu(s*h + b) fused on ACT in T layout ----
                    for t in range(nt):
                        hs = h_sb[:, t * dout:(t + 1) * dout]
                        tp = ps.tile([P, P], f32, tag="tpps", bufs=2,
                                     name="tp")
                        nc.tensor.transpose(out=tp[:dout, :], in_=hs,
                                            identity=ident[:])
                        nc.scalar.activation(xT[:dout, t * P:(t + 1) * P],
                                             tp[:dout, :], ACTF.Relu,
                                             bias=bcol[:dout, :],
                                             scale=scol[:dout, :])
    nc.compile()
    return nc


def prepare(z_nodes, src, dst, edge_weight,
            W0, b0, W1, b1, W2, b2,
            gamma0, beta0, gamma1, beta1):
    """Host-side restructuring + program build; returns (nc, in_maps)."""
    z = np.asarray(z_nodes, np.float32)
    src = np.asarray(src).astype(np.int64)
    dst = np.asarray(dst).astype(np.int64)
    ew = np.asarray(edge_weight, np.float32)
    n_nodes, d_in = z.shape
    d_hid = np.asarray(W0).shape[1]
    assert n_nodes % NCORES == 0

    perm = None
    if math.ceil((n_nodes // NCORES) / P) >= 4:
        perm = _balance_perm(src, dst, n_nodes)
        inv = np.empty_like(perm)
        inv[perm] = np.arange(n_nodes)
        z = z[inv]
        src = perm[src]
        dst = perm[dst]

    shared, per_core = _edge_structure(src, dst, ew, n_nodes)
    nloc, nt = shared["nloc"], shared["ntiles"]

    nc = _build_program(n_nodes, d_in, d_hid, shared)

    iota = np.tile(np.arange(P, dtype=np.float32), (P, 1))
    rmask = np.zeros((P, 1), np.float32)
    nlr = nloc - (nt - 1) * P
    rmask[:nlr] = 1.0
    consts = {
        "rmask": rmask,
        "iotab": _to_bf16(iota),
        "iota": np.ascontiguousarray(iota),
        "ident": np.eye(P, dtype=np.float32),
        "W0": np.asarray(W0, np.float32), "W1": np.asarray(W1, np.float32),
        "W2": np.asarray(W2, np.float32),
        "b2": np.asarray(b2, np.float32).reshape(1, -1),
        "gm0": np.asarray(gamma0, np.float32).reshape(1, -1),
        "gm1": np.asarray(gamma1, np.float32).reshape(1, -1),
        "bt0": np.asarray(beta0, np.float32).reshape(1, -1),
        "bt1": np.asarray(beta1, np.float32).reshape(1, -1),
    }
    in_maps = []
    for c in range(NCORES):
        pc = per_core[c]
        zc = z[c * nloc:(c + 1) * nloc]
        zT = np.zeros((d_in, nt * P), np.float32)
        zT[:, :nloc] = zc.T
        in_maps.append({**consts, "zT": zT, "ewbd": pc["ewbd"],
                        "ewT": pc["ewT"], "dmT": pc["dmT"],
                        "ewTb": _to_bf16(pc["ewT"]),
                        "dmTb": _to_bf16(pc["dmT"]),
                        "idxL": pc["idxL"]})
    return nc, in_maps, perm


def kernel(**inputs):
    global LAST_RESULTS
    nc, in_maps, perm = prepare(**inputs)
    res = run_bass_kernel_spmd(nc, in_maps, core_ids=list(range(NCORES)))
    LAST_RESULTS = res
    out = np.concatenate([res.results[c]["out"] for c in range(NCORES)], 0)
    if perm is not None:
        out = out[perm]
    return out

